# revision 15
# baseline (speedup 1.0000x reference)
"""Location-sensitive attention (Tacotron-style) on 8 TRN2 NeuronCores.

Strategy: data-parallel over batch (8 batches per core), all weights
replicated. The 1D conv over prev attention weights is folded into
W_location on the host (Mfold = W_location @ conv_w[:,0,:], a [128,31]
matrix applied to shifted copies of prev). The encoder tensor is shipped
in fp16 in BOTH layouts (E-on-partitions for the projection matmul,
T-on-partitions for the context reduction) — same total bytes as one
fp32 copy — because the PE contracts only along the partition dim.

Per batch on-device pipeline:
  enc_projT[A,T] (+ loc_projT) accumulate in PSUM  -> tanh(+dec bias)
  -> energy columns via per-T-tile matmuls against v -> exp -> masked
  softmax (no max subtraction: |energy| <= ||v||_1 ~ 9, exp is safe)
  -> context via attn-stationary matmuls over native-layout tiles.
"""

import numpy as np

import concourse.bass as bass
import concourse.tile as tile
from concourse import bacc, mybir
from concourse.bass_utils import run_bass_kernel_spmd
from concourse.masks import make_identity

B, T, E, D = 64, 2048, 512, 1024
A, F, KS, PAD = 128, 32, 31, 15
NCORES = 8
BPC = B // NCORES  # batches per core
EC = E // 128      # encoder-dim chunks
DC = D // 128      # decoder-dim chunks
NT = T // 128      # T tiles of 128
NJ = T // 512      # T chunks of 512 (psum bank width)

f32 = mybir.dt.float32
f16 = mybir.dt.float16


def build_nc():
    nc = bacc.Bacc("TRN2", target_bir_lowering=False, debug=False)

    encT = nc.dram_tensor("encT", [BPC, 128, EC, T], f16, kind="ExternalInput")
    encN = nc.dram_tensor("encN", [BPC, 128, NT, E], f16, kind="ExternalInput")
    prevsh = nc.dram_tensor("prevsh", [BPC, KS, T], f16, kind="ExternalInput")
    decT = nc.dram_tensor("decT", [128, DC * BPC], f32, kind="ExternalInput")
    WencT = nc.dram_tensor("WencT", [128, EC * A], f16, kind="ExternalInput")
    WdecT = nc.dram_tensor("WdecT", [128, DC * A], f32, kind="ExternalInput")
    MlocT = nc.dram_tensor("MlocT", [KS, A], f16, kind="ExternalInput")
    vcol = nc.dram_tensor("vcol", [A, 1], f16, kind="ExternalInput")
    maskc = nc.dram_tensor("maskc", [128, BPC * NT], f32, kind="ExternalInput")
    onesc = nc.dram_tensor("onesc", [128, 1], f32, kind="ExternalInput")
    onesr = nc.dram_tensor("onesr", [1, 128], f32, kind="ExternalInput")
    ctx_out = nc.dram_tensor("ctx_out", [BPC, E], f32, kind="ExternalOutput")
    attn_out = nc.dram_tensor("attn_out", [BPC, T], f32, kind="ExternalOutput")

    with tile.TileContext(nc) as tc:
        with (
            tc.tile_pool(name="consts", bufs=1) as consts,
            tc.tile_pool(name="io", bufs=3) as io,
            tc.tile_pool(name="work", bufs=2) as work,
            tc.tile_pool(name="small", bufs=2) as small,
            tc.tile_pool(name="pe", bufs=4, space="PSUM") as ppe,
            tc.tile_pool(name="pcol", bufs=1, space="PSUM") as pcol,
            tc.tile_pool(name="pctx", bufs=1, space="PSUM") as pctx,
            tc.tile_pool(name="ptiny", bufs=2, space="PSUM") as ptiny,
        ):
            # --- constants ---
            sb_Wenc = consts.tile([128, EC * A], f16)
            nc.scalar.dma_start(out=sb_Wenc[:], in_=WencT[:])
            sb_Mloc = consts.tile([KS, A], f16)
            nc.scalar.dma_start(out=sb_Mloc[:], in_=MlocT[:])
            sb_v = consts.tile([A, 1], f16)
            nc.scalar.dma_start(out=sb_v[:], in_=vcol[:])
            sb_onesc = consts.tile([128, 1], f32)
            nc.scalar.dma_start(out=sb_onesc[:], in_=onesc[:])
            sb_onesr = consts.tile([1, 128], f32)
            nc.scalar.dma_start(out=sb_onesr[:], in_=onesr[:])
            sb_Wdec = consts.tile([128, DC * A], f32)
            nc.gpsimd.dma_start(out=sb_Wdec[:], in_=WdecT[:])
            sb_decT = consts.tile([128, DC * BPC], f32)
            nc.gpsimd.dma_start(out=sb_decT[:], in_=decT[:])
            sb_mask = consts.tile([128, BPC * NT], f32)
            nc.scalar.dma_start(out=sb_mask[:], in_=maskc[:])
            sb_ident = consts.tile([128, 128], f32)
            make_identity(nc, sb_ident[:])

            sb_dec = consts.tile([128, BPC], f32)

            for b in range(BPC):
                # --- loads ---
                eTs = [
                    io.tile([128, T], f16, tag="encT", bufs=8, name=f"eT_b{b}_c{c}")
                    for c in range(EC)
                ]
                for c in range(EC):
                    nc.sync.dma_start(out=eTs[c][:], in_=encT[b, :, c, :])
                sb_eN = io.tile([128, NT * E], f16, tag="encN")
                nc.gpsimd.dma_start(
                    out=sb_eN[:], in_=encN[b].rearrange("p i e -> p (i e)")
                )
                sb_pv = io.tile([KS, T], f16, tag="prev")
                nc.scalar.dma_start(out=sb_pv[:], in_=prevsh[b])

                # --- enc_projT + loc_projT -> psum [A, T] in 4 bank chunks ---
                chunks = [
                    ppe.tile([128, 512], f32, tag="pe", name=f"pe_b{b}_j{j}")
                    for j in range(NJ)
                ]
                def proj_mm(c, j):
                    nc.tensor.matmul(
                        chunks[j][:],
                        lhsT=sb_Wenc[:, c * A:(c + 1) * A],
                        rhs=eTs[c][:, j * 512:(j + 1) * 512],
                        start=(c == 0),
                        stop=False,
                    )

                def loc_mm(j):
                    nc.tensor.matmul(
                        chunks[j][:],
                        lhsT=sb_Mloc[:],
                        rhs=sb_pv[:, j * 512:(j + 1) * 512],
                        start=False,
                        stop=True,
                    )

                if b == BPC - 1:
                    # last batch: finish each psum bank asap so the tanh ->
                    # energy -> softmax -> context drain chain starts early
                    for j in range(NJ):
                        for c in range(EC):
                            proj_mm(c, j)
                        loc_mm(j)
                else:
                    # steady state: keep the stationary weight hot across j
                    for c in range(EC):
                        for j in range(NJ):
                            proj_mm(c, j)
                    for j in range(NJ):
                        loc_mm(j)

                if b == 0:
                    # dec_proj[A, :] for all local batches (tiny, fp32);
                    # emitted after batch 0's projection so PE starts on the
                    # encoder stream immediately
                    psum_dec = ptiny.tile([128, BPC], f32, tag="tiny")
                    for k in range(DC):
                        nc.tensor.matmul(
                            psum_dec[:],
                            lhsT=sb_Wdec[:, k * A:(k + 1) * A],
                            rhs=sb_decT[:, k * BPC:(k + 1) * BPC],
                            start=(k == 0),
                            stop=(k == DC - 1),
                        )
                    nc.vector.tensor_copy(sb_dec[:], psum_dec[:])

                # --- tanh(psum + dec_proj[:, b]) -> fp16, per 512-col chunk ---
                tanhs = [
                    work.tile([128, 512], f16, tag="tanh", bufs=8,
                              name=f"tanh_b{b}_j{j}")
                    for j in range(NJ)
                ]
                for j in range(NJ):
                    nc.scalar.activation(
                        out=tanhs[j][:],
                        in_=chunks[j][:],
                        func=mybir.ActivationFunctionType.Tanh,
                        bias=sb_dec[:, b:b + 1],
                    )

                # --- energy columns: e[t] = v . tanh[:, t], per T tile ---
                psum_ecol = pcol.tile([128, NT], f32, tag="ecol")
                for i in range(NT):
                    nc.tensor.matmul(
                        psum_ecol[:, i:i + 1],
                        lhsT=tanhs[i // 4][:, (i % 4) * 128:(i % 4 + 1) * 128],
                        rhs=sb_v[:],
                        start=True,
                        stop=True,
                    )

                # --- exp, mask, row sum ---
                sb_exp = small.tile([128, NT], f32, tag="exp")
                nc.scalar.activation(
                    out=sb_exp[:], in_=psum_ecol[:],
                    func=mybir.ActivationFunctionType.Exp,
                )
                sb_expm = small.tile([128, NT], f32, tag="expm")
                nc.vector.tensor_mul(
                    sb_expm[:], sb_exp[:], sb_mask[:, b * NT:(b + 1) * NT]
                )
                sb_sum = small.tile([128, 1], f32, tag="sum")
                nc.vector.reduce_sum(sb_sum[:], sb_expm[:], axis=mybir.AxisListType.X)

                # --- S = total sum; 1/S broadcast to all partitions ---
                psum_s = ptiny.tile([1, 1], f32, tag="tiny")
                nc.tensor.matmul(
                    psum_s[:], lhsT=sb_onesc[:], rhs=sb_sum[:], start=True, stop=True
                )
                sb_rcp = small.tile([1, 1], f32, tag="rcp")
                nc.vector.reciprocal(sb_rcp[:], psum_s[:])
                psum_bc = ptiny.tile([128, 1], f32, tag="tiny")
                nc.tensor.matmul(
                    psum_bc[:], lhsT=sb_onesr[:], rhs=sb_rcp[:], start=True, stop=True
                )
                sb_rcpb = small.tile([128, 1], f32, tag="rcpb")
                nc.vector.tensor_copy(sb_rcpb[:], psum_bc[:])

                # --- attention weights (cols) in fp32 and fp16 ---
                sb_attn = small.tile([128, NT], f32, tag="attn")
                nc.vector.tensor_scalar_mul(sb_attn[:], sb_expm[:], sb_rcpb[:])
                sb_attn16 = small.tile([128, NT], f16, tag="attn16")
                nc.vector.tensor_copy(sb_attn16[:], sb_attn[:])

                # --- attention weights as a row (transpose [128,NT]->[NT,128]) ---
                psum_at = ptiny.tile([NT, 128], f32, tag="tiny")
                nc.tensor.transpose(psum_at[:], sb_attn[:], sb_ident[:])
                sb_arow = small.tile([NT, 128], f32, tag="arow")
                nc.vector.tensor_copy(sb_arow[:], psum_at[:])
                nc.sync.dma_start(
                    out=attn_out[b].rearrange("(i p) -> i p", p=128), in_=sb_arow[:]
                )

                # --- context: ctx[e] = sum_t attn[t] * enc[t, e] ---
                psum_ctx = pctx.tile([1, E], f32, tag="ctx")
                for i in range(NT):
                    nc.tensor.matmul(
                        psum_ctx[:],
                        lhsT=sb_attn16[:, i:i + 1],
                        rhs=sb_eN[:, i * E:(i + 1) * E],
                        start=(i == 0),
                        stop=(i == NT - 1),
                    )
                sb_ctx = small.tile([1, E], f32, tag="ctx")
                nc.vector.tensor_copy(sb_ctx[:], psum_ctx[:])
                nc.sync.dma_start(out=ctx_out[b][None, :], in_=sb_ctx[:])


    nc.compile()
    return nc


_NC = None


def _get_nc():
    global _NC
    if _NC is None:
        _NC = build_nc()
    return _NC


def _prep_in_maps(decoder_state, encoder_outputs, prev_attention_weights,
                  encoder_mask, conv_w, W_encoder, W_decoder, W_location, v):
    dec = np.asarray(decoder_state, np.float32)
    enc = np.asarray(encoder_outputs, np.float32)
    prev = np.asarray(prev_attention_weights, np.float32)
    mask = np.asarray(encoder_mask)
    conv_w = np.asarray(conv_w, np.float32)
    W_encoder = np.asarray(W_encoder, np.float32)
    W_decoder = np.asarray(W_decoder, np.float32)
    W_location = np.asarray(W_location, np.float32)
    v = np.asarray(v, np.float32)

    # fold conv into the location projection: [A, KS]
    Mfold = W_location @ conv_w[:, 0, :]
    prevpad = np.pad(prev, ((0, 0), (PAD, PAD)))
    idx = np.arange(KS)[:, None] + np.arange(T)[None, :]
    shifted = prevpad[:, idx].astype(np.float16)  # [B, KS, T]

    enc16 = enc.astype(np.float16)
    # encN_h[b, p, i, e] = enc[b, i*128+p, e]  (native tiles, T on partitions)
    encN_h = np.ascontiguousarray(enc16.reshape(B, NT, 128, E).transpose(0, 2, 1, 3))
    # encT_h[b, p, c, t] = enc[b, t, c*128+p]  (transposed tiles, E on partitions)
    encT_h = np.ascontiguousarray(
        enc16.transpose(0, 2, 1).reshape(B, EC, 128, T).transpose(0, 2, 1, 3)
    )

    WencT_h = np.ascontiguousarray(
        W_encoder.T.reshape(EC, 128, A).transpose(1, 0, 2).reshape(128, EC * A)
    ).astype(np.float16)
    WdecT_h = np.ascontiguousarray(
        W_decoder.T.reshape(DC, 128, A).transpose(1, 0, 2).reshape(128, DC * A)
    ).astype(np.float32)
    MlocT_h = np.ascontiguousarray(Mfold.T).astype(np.float16)
    vcol_h = v.reshape(A, 1).astype(np.float16)
    onesc_h = np.ones((128, 1), np.float32)
    onesr_h = np.ones((1, 128), np.float32)

    in_maps = []
    for core in range(NCORES):
        sl = slice(core * BPC, (core + 1) * BPC)
        in_maps.append({
            "encT": encT_h[sl],
            "encN": encN_h[sl],
            "prevsh": np.ascontiguousarray(shifted[sl]),
            "decT": np.ascontiguousarray(
                dec[sl].T.reshape(DC, 128, BPC).transpose(1, 0, 2).reshape(128, DC * BPC)
            ),
            "WencT": WencT_h,
            "WdecT": WdecT_h,
            "MlocT": MlocT_h,
            "vcol": vcol_h,
            "maskc": np.ascontiguousarray(
                mask[sl].reshape(BPC, NT, 128).transpose(2, 0, 1).reshape(128, BPC * NT)
            ).astype(np.float32),
            "onesc": onesc_h,
            "onesr": onesr_h,
        })
    return in_maps


def kernel(**inputs):
    in_maps = _prep_in_maps(**inputs)
    nc = _get_nc()
    res = run_bass_kernel_spmd(nc, in_maps, list(range(NCORES))).results
    ctx = np.concatenate([r["ctx_out"] for r in res], 0).astype(np.float32)
    attn = np.concatenate([r["attn_out"] for r in res], 0).astype(np.float32)
    return ctx, attn


# revision 16
# speedup vs baseline: 1.0012x; 1.0012x over previous
"""Location-sensitive attention (Tacotron-style) on 8 TRN2 NeuronCores.

Strategy: data-parallel over batch (8 batches per core), all weights
replicated. The 1D conv over prev attention weights is folded into
W_location on the host (Mfold = W_location @ conv_w[:,0,:], a [128,31]
matrix applied to shifted copies of prev). The encoder tensor is shipped
in fp16 in BOTH layouts (E-on-partitions for the projection matmul,
T-on-partitions for the context reduction) — same total bytes as one
fp32 copy — because the PE contracts only along the partition dim.

Per batch on-device pipeline:
  enc_projT[A,T] (+ loc_projT) accumulate in PSUM  -> tanh(+dec bias)
  -> energy columns via per-T-tile matmuls against v -> exp -> masked
  softmax (no max subtraction: |energy| <= ||v||_1 ~ 9, exp is safe)
  -> context via attn-stationary matmuls over native-layout tiles.
"""

import numpy as np

import concourse.bass as bass
import concourse.tile as tile
from concourse import bacc, mybir
from concourse.bass_utils import run_bass_kernel_spmd
from concourse.masks import make_identity

B, T, E, D = 64, 2048, 512, 1024
A, F, KS, PAD = 128, 32, 31, 15
NCORES = 8
BPC = B // NCORES  # batches per core
EC = E // 128      # encoder-dim chunks
DC = D // 128      # decoder-dim chunks
NT = T // 128      # T tiles of 128
NJ = T // 512      # T chunks of 512 (psum bank width)

f32 = mybir.dt.float32
f16 = mybir.dt.float16


def build_nc():
    nc = bacc.Bacc("TRN2", target_bir_lowering=False, debug=False)

    encT = nc.dram_tensor("encT", [BPC, 128, EC, T], f16, kind="ExternalInput")
    encN = nc.dram_tensor("encN", [BPC, 128, NT, E], f16, kind="ExternalInput")
    prevsh = nc.dram_tensor("prevsh", [BPC, KS, T], f16, kind="ExternalInput")
    decT = nc.dram_tensor("decT", [128, DC * BPC], f32, kind="ExternalInput")
    WencT = nc.dram_tensor("WencT", [128, EC * A], f16, kind="ExternalInput")
    WdecT = nc.dram_tensor("WdecT", [128, DC * A], f32, kind="ExternalInput")
    MlocT = nc.dram_tensor("MlocT", [KS, A], f16, kind="ExternalInput")
    vcol = nc.dram_tensor("vcol", [A, 1], f16, kind="ExternalInput")
    maskc = nc.dram_tensor("maskc", [128, BPC * NT], f32, kind="ExternalInput")
    onesc = nc.dram_tensor("onesc", [128, 1], f32, kind="ExternalInput")
    onesr = nc.dram_tensor("onesr", [1, 128], f32, kind="ExternalInput")
    ctx_out = nc.dram_tensor("ctx_out", [BPC, E], f32, kind="ExternalOutput")
    attn_out = nc.dram_tensor("attn_out", [BPC, T], f32, kind="ExternalOutput")

    with tile.TileContext(nc) as tc:
        with (
            tc.tile_pool(name="consts", bufs=1) as consts,
            tc.tile_pool(name="io", bufs=3) as io,
            tc.tile_pool(name="work", bufs=2) as work,
            tc.tile_pool(name="small", bufs=2) as small,
            tc.tile_pool(name="pe", bufs=4, space="PSUM") as ppe,
            tc.tile_pool(name="pcol", bufs=1, space="PSUM") as pcol,
            tc.tile_pool(name="pctx", bufs=1, space="PSUM") as pctx,
            tc.tile_pool(name="ptiny", bufs=2, space="PSUM") as ptiny,
        ):
            # --- constants ---
            sb_Wenc = consts.tile([128, EC * A], f16)
            nc.scalar.dma_start(out=sb_Wenc[:], in_=WencT[:])
            sb_Mloc = consts.tile([KS, A], f16)
            nc.scalar.dma_start(out=sb_Mloc[:], in_=MlocT[:])
            sb_v = consts.tile([A, 1], f16)
            nc.scalar.dma_start(out=sb_v[:], in_=vcol[:])
            sb_onesc = consts.tile([128, 1], f32)
            nc.scalar.dma_start(out=sb_onesc[:], in_=onesc[:])
            sb_onesr = consts.tile([1, 128], f32)
            nc.scalar.dma_start(out=sb_onesr[:], in_=onesr[:])
            sb_Wdec = consts.tile([128, DC * A], f32)
            nc.gpsimd.dma_start(out=sb_Wdec[:], in_=WdecT[:])
            sb_decT = consts.tile([128, DC * BPC], f32)
            nc.gpsimd.dma_start(out=sb_decT[:], in_=decT[:])
            sb_mask = consts.tile([128, BPC * NT], f32)
            nc.scalar.dma_start(out=sb_mask[:], in_=maskc[:])
            sb_ident = consts.tile([128, 128], f32)
            make_identity(nc, sb_ident[:])

            sb_dec = consts.tile([128, BPC], f32)

            for b in range(BPC):
                # --- loads ---
                eTs = [
                    io.tile([128, T], f16, tag="encT", bufs=8, name=f"eT_b{b}_c{c}")
                    for c in range(EC)
                ]
                for c in range(EC):
                    nc.sync.dma_start(out=eTs[c][:], in_=encT[b, :, c, :])
                sb_eN = io.tile([128, NT * E], f16, tag="encN")
                nc.gpsimd.dma_start(
                    out=sb_eN[:], in_=encN[b].rearrange("p i e -> p (i e)")
                )
                sb_pv = io.tile([KS, T], f16, tag="prev")
                nc.scalar.dma_start(out=sb_pv[:], in_=prevsh[b])

                # --- enc_projT + loc_projT -> psum [A, T] in 4 bank chunks ---
                chunks = [
                    ppe.tile([128, 512], f32, tag="pe", name=f"pe_b{b}_j{j}")
                    for j in range(NJ)
                ]
                def proj_mm(c, j):
                    nc.tensor.matmul(
                        chunks[j][:],
                        lhsT=sb_Wenc[:, c * A:(c + 1) * A],
                        rhs=eTs[c][:, j * 512:(j + 1) * 512],
                        start=(c == 0),
                        stop=False,
                    )

                def loc_mm(j):
                    nc.tensor.matmul(
                        chunks[j][:],
                        lhsT=sb_Mloc[:],
                        rhs=sb_pv[:, j * 512:(j + 1) * 512],
                        start=False,
                        stop=True,
                    )

                if b == BPC - 1:
                    # last batch: finish each psum bank asap so the tanh ->
                    # energy -> softmax -> context drain chain starts early
                    for j in range(NJ):
                        for c in range(EC):
                            proj_mm(c, j)
                        loc_mm(j)
                else:
                    # steady state: keep the stationary weight hot across j
                    for c in range(EC):
                        for j in range(NJ):
                            proj_mm(c, j)
                    for j in range(NJ):
                        loc_mm(j)

                if b == 0:
                    # dec_proj[A, :] for all local batches (tiny, fp32);
                    # emitted after batch 0's projection so PE starts on the
                    # encoder stream immediately
                    psum_dec = ptiny.tile([128, BPC], f32, tag="tiny")
                    for k in range(DC):
                        nc.tensor.matmul(
                            psum_dec[:],
                            lhsT=sb_Wdec[:, k * A:(k + 1) * A],
                            rhs=sb_decT[:, k * BPC:(k + 1) * BPC],
                            start=(k == 0),
                            stop=(k == DC - 1),
                        )
                    nc.vector.tensor_copy(sb_dec[:], psum_dec[:])

                # --- tanh(psum + dec_proj[:, b]) -> fp16, per 512-col chunk ---
                tanhs = [
                    work.tile([128, 512], f16, tag="tanh", bufs=8,
                              name=f"tanh_b{b}_j{j}")
                    for j in range(NJ)
                ]
                for j in range(NJ):
                    nc.scalar.activation(
                        out=tanhs[j][:],
                        in_=chunks[j][:],
                        func=mybir.ActivationFunctionType.Tanh,
                        bias=sb_dec[:, b:b + 1],
                    )

                # --- energy columns: e[t] = v . tanh[:, t], per T tile ---
                psum_ecol = pcol.tile([128, NT], f32, tag="ecol")
                for i in range(NT):
                    nc.tensor.matmul(
                        psum_ecol[:, i:i + 1],
                        lhsT=tanhs[i // 4][:, (i % 4) * 128:(i % 4 + 1) * 128],
                        rhs=sb_v[:],
                        start=True,
                        stop=True,
                    )

                # --- additive mask, then exp with fused row-sum ---
                sb_emask = small.tile([128, NT], f32, tag="emask")
                nc.vector.tensor_add(
                    sb_emask[:], psum_ecol[:], sb_mask[:, b * NT:(b + 1) * NT]
                )
                sb_expm = small.tile([128, NT], f32, tag="expm")
                sb_sum = small.tile([128, 1], f32, tag="sum")
                nc.scalar.activation(
                    out=sb_expm[:], in_=sb_emask[:],
                    func=mybir.ActivationFunctionType.Exp,
                    accum_out=sb_sum[:],
                )

                # --- S = total sum; 1/S broadcast to all partitions ---
                psum_s = ptiny.tile([1, 1], f32, tag="tiny")
                nc.tensor.matmul(
                    psum_s[:], lhsT=sb_onesc[:], rhs=sb_sum[:], start=True, stop=True
                )
                sb_rcp = small.tile([1, 1], f32, tag="rcp")
                nc.vector.reciprocal(sb_rcp[:], psum_s[:])
                psum_bc = ptiny.tile([128, 1], f32, tag="tiny")
                nc.tensor.matmul(
                    psum_bc[:], lhsT=sb_onesr[:], rhs=sb_rcp[:], start=True, stop=True
                )
                sb_rcpb = small.tile([128, 1], f32, tag="rcpb")
                nc.vector.tensor_copy(sb_rcpb[:], psum_bc[:])

                # --- attention weights (cols) in fp32 and fp16 ---
                sb_attn = small.tile([128, NT], f32, tag="attn")
                nc.vector.tensor_scalar_mul(sb_attn[:], sb_expm[:], sb_rcpb[:])
                sb_attn16 = small.tile([128, NT], f16, tag="attn16")
                nc.vector.tensor_copy(sb_attn16[:], sb_attn[:])

                # --- attention weights as a row (transpose [128,NT]->[NT,128]) ---
                psum_at = ptiny.tile([NT, 128], f32, tag="tiny")
                nc.tensor.transpose(psum_at[:], sb_attn[:], sb_ident[:])
                sb_arow = small.tile([NT, 128], f32, tag="arow")
                nc.vector.tensor_copy(sb_arow[:], psum_at[:])
                nc.sync.dma_start(
                    out=attn_out[b].rearrange("(i p) -> i p", p=128), in_=sb_arow[:]
                )

                # --- context: ctx[e] = sum_t attn[t] * enc[t, e] ---
                psum_ctx = pctx.tile([1, E], f32, tag="ctx")
                for i in range(NT):
                    nc.tensor.matmul(
                        psum_ctx[:],
                        lhsT=sb_attn16[:, i:i + 1],
                        rhs=sb_eN[:, i * E:(i + 1) * E],
                        start=(i == 0),
                        stop=(i == NT - 1),
                    )
                sb_ctx = small.tile([1, E], f32, tag="ctx")
                nc.vector.tensor_copy(sb_ctx[:], psum_ctx[:])
                nc.sync.dma_start(out=ctx_out[b][None, :], in_=sb_ctx[:])


    nc.compile()
    return nc


_NC = None


def _get_nc():
    global _NC
    if _NC is None:
        _NC = build_nc()
    return _NC


def _prep_in_maps(decoder_state, encoder_outputs, prev_attention_weights,
                  encoder_mask, conv_w, W_encoder, W_decoder, W_location, v):
    dec = np.asarray(decoder_state, np.float32)
    enc = np.asarray(encoder_outputs, np.float32)
    prev = np.asarray(prev_attention_weights, np.float32)
    mask = np.asarray(encoder_mask)
    conv_w = np.asarray(conv_w, np.float32)
    W_encoder = np.asarray(W_encoder, np.float32)
    W_decoder = np.asarray(W_decoder, np.float32)
    W_location = np.asarray(W_location, np.float32)
    v = np.asarray(v, np.float32)

    # fold conv into the location projection: [A, KS]
    Mfold = W_location @ conv_w[:, 0, :]
    prevpad = np.pad(prev, ((0, 0), (PAD, PAD)))
    idx = np.arange(KS)[:, None] + np.arange(T)[None, :]
    shifted = prevpad[:, idx].astype(np.float16)  # [B, KS, T]

    enc16 = enc.astype(np.float16)
    # encN_h[b, p, i, e] = enc[b, i*128+p, e]  (native tiles, T on partitions)
    encN_h = np.ascontiguousarray(enc16.reshape(B, NT, 128, E).transpose(0, 2, 1, 3))
    # encT_h[b, p, c, t] = enc[b, t, c*128+p]  (transposed tiles, E on partitions)
    encT_h = np.ascontiguousarray(
        enc16.transpose(0, 2, 1).reshape(B, EC, 128, T).transpose(0, 2, 1, 3)
    )

    WencT_h = np.ascontiguousarray(
        W_encoder.T.reshape(EC, 128, A).transpose(1, 0, 2).reshape(128, EC * A)
    ).astype(np.float16)
    WdecT_h = np.ascontiguousarray(
        W_decoder.T.reshape(DC, 128, A).transpose(1, 0, 2).reshape(128, DC * A)
    ).astype(np.float32)
    MlocT_h = np.ascontiguousarray(Mfold.T).astype(np.float16)
    vcol_h = v.reshape(A, 1).astype(np.float16)
    onesc_h = np.ones((128, 1), np.float32)
    onesr_h = np.ones((1, 128), np.float32)

    in_maps = []
    for core in range(NCORES):
        sl = slice(core * BPC, (core + 1) * BPC)
        in_maps.append({
            "encT": encT_h[sl],
            "encN": encN_h[sl],
            "prevsh": np.ascontiguousarray(shifted[sl]),
            "decT": np.ascontiguousarray(
                dec[sl].T.reshape(DC, 128, BPC).transpose(1, 0, 2).reshape(128, DC * BPC)
            ),
            "WencT": WencT_h,
            "WdecT": WdecT_h,
            "MlocT": MlocT_h,
            "vcol": vcol_h,
            "maskc": np.ascontiguousarray(
                (mask[sl].reshape(BPC, NT, 128) == 0).transpose(2, 0, 1)
                .reshape(128, BPC * NT)
            ).astype(np.float32) * np.float32(-1e9),
            "onesc": onesc_h,
            "onesr": onesr_h,
        })
    return in_maps


def kernel(**inputs):
    in_maps = _prep_in_maps(**inputs)
    nc = _get_nc()
    res = run_bass_kernel_spmd(nc, in_maps, list(range(NCORES))).results
    ctx = np.concatenate([r["ctx_out"] for r in res], 0).astype(np.float32)
    attn = np.concatenate([r["attn_out"] for r in res], 0).astype(np.float32)
    return ctx, attn
